# revision 8
# baseline (speedup 1.0000x reference)
"""2-layer GAT on 8 trn2 NeuronCores — v3.

Same math as the staged baseline, restructured for fewer DMA instructions:
 - Host: degree-sort nodes; position space of 98*1024 slots; block j*8+c
   belongs to core c. One per-core index matrix [128, sum(K_j)] i32 (blocks
   side by side in columns, self loop at slot 0, pad -> sentinel row) serves
   BOTH layers (both tables in position space, sentinel appended at the end).
 - prog1: H1 table [GSLOTS+1, 68] bf16 built in 2048-node groups (1 load +
   1 interleaved write per group), index matrix preloaded to SBUF in ONE DMA,
   per-block gathers via per-slot [P,1] indirect DMA (the only ucode-correct
   form), softmax-weighted aggregation per partition, W2 projection, h2 shard
   written partition-major [128, 98*36].
 - Host reassembles the full layer-2 table (free), appends sentinel row.
 - prog2: same aggregation + final row softmax, output [128, 98*32] f32.
"""

import sys
from contextlib import ExitStack

import numpy as np

sys.path.insert(0, "/opt/trn_rl_repo")

import ml_dtypes  # noqa: E402

import concourse.bass as bass  # noqa: E402
import concourse.bacc as bacc  # noqa: E402
import concourse.tile as tile  # noqa: E402
from concourse import mybir  # noqa: E402
from concourse.bass_utils import run_bass_kernel_spmd  # noqa: E402
from concourse.masks import make_identity  # noqa: E402

N = 100000
E = 3200000
IN_F, HID_F, OUT_F = 128, 64, 32
NEG = 0.2
CORES = 8
P = 128
NBLK = 98
GSLOTS = NBLK * CORES * P   # 100352 global position slots
SENT = GSLOTS               # sentinel row in both tables
E1 = HID_F + 4              # 68 bf16: h(64) | gs f32 | gd f32
E2 = OUT_F + 4              # 36 bf16
T1 = E1 // 2
T2 = E2 // 2

bf = mybir.dt.bfloat16
f32 = mybir.dt.float32
i32 = mybir.dt.int32
AF = mybir.ActivationFunctionType
OP = mybir.AluOpType
AX = mybir.AxisListType

LAST_RESULT = None
_CACHE = {}


# ----------------------------------------------------------------- host prep
def _host_prep(edge_index):
    src = np.asarray(edge_index[0], dtype=np.int64)
    dst = np.asarray(edge_index[1], dtype=np.int64)
    deg = np.bincount(dst, minlength=N).astype(np.int64) + 1
    order = np.argsort(-deg, kind="stable")
    degs = deg[order]
    Ks = [int(degs[j * CORES * P]) for j in range(NBLK)]
    pos_of_node = np.empty(N, dtype=np.int64)
    pos_of_node[order] = np.arange(N)

    eorder = np.argsort(dst, kind="stable")
    ss = src[eorder]
    ds = dst[eorder]
    counts = np.bincount(dst, minlength=N)
    starts = np.zeros(N, dtype=np.int64)
    starts[1:] = np.cumsum(counts)[:-1]

    Kmax = Ks[0]
    Mpos = np.full((GSLOTS, Kmax), SENT, dtype=np.int32)
    Mpos[:N, 0] = np.arange(N, dtype=np.int32)
    slot_k = (np.arange(E) - starts[ds] + 1).astype(np.int64)
    Mpos[pos_of_node[ds], slot_k] = pos_of_node[ss].astype(np.int32)

    Mv = Mpos.reshape(NBLK, CORES, P, Kmax)
    idx = np.empty((CORES, P, sum(Ks)), dtype=np.int32)
    for c in range(CORES):
        idx[c] = np.concatenate(
            [Mv[j, c, :, :Ks[j]] for j in range(NBLK)], axis=1)
    return Ks, order, idx


# ------------------------------------------------------------- device pieces
def _mk_common(nc, tc, ctx, SUMK, ixd):
    consts = ctx.enter_context(tc.tile_pool(name="consts", bufs=1))
    psum = ctx.enter_context(tc.tile_pool(name="psum", bufs=2, space="PSUM"))
    ones1 = consts.tile([1, P], bf)
    nc.gpsimd.memset(ones1[:], 1.0)
    ixsb = consts.tile([P, SUMK], i32)
    nc.sync.dma_start(out=ixsb[:], in_=ixd.ap())
    return consts, psum, ones1, ixsb


def _rep_bias(nc, consts, psum, ones1, bd, fw, tag):
    br = consts.tile([1, fw], bf)
    nc.sync.dma_start(out=br[:], in_=bd.ap())
    brep = consts.tile([P, fw], f32)
    pb = psum.tile([P, fw], f32, tag=tag)
    nc.tensor.matmul(out=pb[:], lhsT=ones1[:], rhs=br[:], start=True, stop=True)
    nc.vector.tensor_copy(out=brep[:], in_=pb[:])
    return brep


def _agg_layer(nc, sb, psum, Ks, ixsb, table_ap, ew, fw, tw, brep,
               wnext, ident, outdram):
    cum = np.zeros(NBLK + 1, dtype=np.int64)
    cum[1:] = np.cumsum(Ks)
    Kmax = Ks[0]
    for j in range(NBLK):
        K = Ks[j]
        c0 = int(cum[j])
        hg = sb.tile([P, K * ew], bf, tag="hg", padded_shape=[P, Kmax * ew])
        for k in range(K):
            gi = nc.gpsimd.indirect_dma_start(
                out=hg[:, k * ew:(k + 1) * ew], out_offset=None,
                in_=table_ap,
                in_offset=bass.IndirectOffsetOnAxis(
                    ap=ixsb[:, c0 + k:c0 + k + 1], axis=0))
            if k % 2:
                gi.ins.queue = "qPoolDynamic1"
        hgf = hg[:].bitcast(f32)
        z = sb.tile([P, K], f32, tag="z", padded_shape=[P, Kmax])
        nc.vector.tensor_scalar(
            out=z[:].rearrange("p (k o) -> p k o", o=1),
            in0=hgf.rearrange("p (k t) -> p k t", t=tw)[:, :, tw - 2:tw - 1],
            scalar1=hgf[:, tw - 1:tw], scalar2=None, op0=OP.add)
        zl = sb.tile([P, K], f32, tag="zl", padded_shape=[P, Kmax])
        nc.vector.scalar_tensor_tensor(
            out=zl[:], in0=z[:], scalar=NEG, in1=z[:],
            op0=OP.mult, op1=OP.max)
        ez = sb.tile([P, K], bf, tag="ez", padded_shape=[P, Kmax])
        den = sb.tile([P, 1], f32, tag="den")
        nc.scalar.activation(out=ez[:], in_=zl[:], func=AF.Exp,
                             accum_out=den[:])
        r = sb.tile([P, 1], f32, tag="r")
        nc.vector.reciprocal(out=r[:], in_=den[:])
        tmp = sb.tile([P, fw * K], bf, tag="tmp", padded_shape=[P, fw * Kmax])
        hg3 = hg[:].rearrange("p (k e) -> p e k", e=ew)[:, 0:fw, :]
        ez3 = ez[:].rearrange("p (k o) -> p o k", o=1).to_broadcast([P, fw, K])
        nc.vector.tensor_tensor(
            out=tmp[:].rearrange("p (f k) -> p f k", k=K),
            in0=hg3, in1=ez3, op=OP.mult)
        num = sb.tile([P, fw], f32, tag="num")
        nc.vector.tensor_reduce(
            out=num[:], in_=tmp[:].rearrange("p (f k) -> p f k", k=K),
            axis=AX.X, op=OP.add)
        o1 = sb.tile([P, fw], f32, tag="o1")
        nc.vector.scalar_tensor_tensor(
            out=o1[:], in0=num[:], scalar=r[:], in1=brep[:],
            op0=OP.mult, op1=OP.add)
        if wnext is not None:
            o1b = sb.tile([P, fw], bf, tag="o1b")
            nc.scalar.activation(out=o1b[:], in_=o1[:], func=AF.Relu)
            pt = psum.tile([fw, P], bf, tag="pt")
            nc.tensor.transpose(out=pt[:], in_=o1b[:], identity=ident[:])
            o1T = sb.tile([fw, P], bf, tag="o1T")
            nc.scalar.activation(out=o1T[:], in_=pt[:], func=AF.Copy)
            p34 = psum.tile([P, OUT_F + 2], f32, tag="p34")
            nc.tensor.matmul(out=p34[:], lhsT=o1T[:], rhs=wnext[:],
                             start=True, stop=True)
            th2 = sb.tile([P, E2], bf, tag="th2")
            nc.scalar.activation(out=th2[:, 0:OUT_F], in_=p34[:, 0:OUT_F],
                                 func=AF.Copy)
            nc.vector.tensor_copy(
                out=th2[:, OUT_F:OUT_F + 4].bitcast(f32),
                in_=p34[:, OUT_F:OUT_F + 2])
            nc.sync.dma_start(out=outdram.ap()[:, j * E2:(j + 1) * E2],
                              in_=th2[:])
        else:
            negm = sb.tile([P, 1], f32, tag="negm")
            nc.vector.tensor_reduce(out=negm[:], in_=o1[:], axis=AX.X,
                                    op=OP.max, negate=True)
            e2 = sb.tile([P, fw], f32, tag="e2")
            nc.scalar.activation(out=e2[:], in_=o1[:], func=AF.Exp,
                                 bias=negm[:])
            ssum = sb.tile([P, 1], f32, tag="ssum")
            nc.vector.tensor_reduce(out=ssum[:], in_=e2[:], axis=AX.X,
                                    op=OP.add)
            rs = sb.tile([P, 1], f32, tag="rs")
            nc.vector.reciprocal(out=rs[:], in_=ssum[:])
            of = sb.tile([P, fw], f32, tag="of")
            nc.vector.tensor_scalar(out=of[:], in0=e2[:], scalar1=rs[:],
                                    scalar2=None, op0=OP.mult)
            nc.sync.dma_start(out=outdram.ap()[:, j * fw:(j + 1) * fw],
                              in_=of[:])


def _build_nc1(Ks):
    SUMK = sum(Ks)
    nc = bacc.Bacc("TRN2", target_bir_lowering=False, debug=False,
                   enable_asserts=False, num_devices=CORES,
                   num_swdge_queues=2)
    xT = nc.dram_tensor("xt", [IN_F, GSLOTS], bf, kind="ExternalInput")
    w1e = nc.dram_tensor("w1e", [IN_F, HID_F + 2], bf, kind="ExternalInput")
    w2e = nc.dram_tensor("w2e", [HID_F, OUT_F + 2], bf, kind="ExternalInput")
    b1d = nc.dram_tensor("b1d", [1, HID_F], bf, kind="ExternalInput")
    ixd = nc.dram_tensor("ixd", [P, SUMK], i32, kind="ExternalInput")
    h2lo = nc.dram_tensor("h2lo", [P, NBLK * E2], bf, kind="ExternalOutput")

    with ExitStack() as ctx:
        tc = ctx.enter_context(tile.TileContext(nc))
        dram = ctx.enter_context(tc.tile_pool(name="dram", bufs=1, space="DRAM"))
        H1 = dram.tile([GSLOTS + 1, E1], bf)
        consts, psum, ones1, ixsb = _mk_common(nc, tc, ctx, SUMK, ixd)
        ident = consts.tile([P, P], bf)
        make_identity(nc, ident[:])
        w1sb = consts.tile([IN_F, HID_F + 2], bf)
        nc.sync.dma_start(out=w1sb[:], in_=w1e.ap())
        w2sb = consts.tile([HID_F, OUT_F + 2], bf)
        nc.sync.dma_start(out=w2sb[:], in_=w2e.ap())
        b1rep = _rep_bias(nc, consts, psum, ones1, b1d, HID_F, "pb1")

        s1 = consts.tile([1, E1], bf)
        nc.gpsimd.memset(s1[:], 0.0)
        nc.gpsimd.memset(s1[:, HID_F:HID_F + 4].bitcast(f32), -1e30)
        nc.sync.dma_start(out=H1[GSLOTS:GSLOTS + 1, :], in_=s1[:])

        sb = ctx.enter_context(tc.tile_pool(name="sb", bufs=3))

        NGRP = GSLOTS // 2048  # 49
        for g in range(NGRP):
            n0 = g * 2048
            xt_t = sb.tile([IN_F, 2048], bf, tag="xt")
            nc.sync.dma_start(out=xt_t[:], in_=xT.ap()[:, n0:n0 + 2048])
            tb = sb.tile([P, 16 * E1], bf, tag="tb")
            tbf = tb[:].bitcast(f32)
            for qq in range(4):
                p4 = psum.tile([P, 4 * (HID_F + 2)], f32, tag="p4")
                for qi in range(4):
                    q = qq * 4 + qi
                    nc.tensor.matmul(
                        out=p4[:, qi * 66:(qi + 1) * 66],
                        lhsT=xt_t[:, q * P:(q + 1) * P],
                        rhs=w1sb[:], start=True, stop=True)
                nc.scalar.activation(
                    out=tb[:].rearrange("p (q e) -> p q e", e=E1)[
                        :, qq * 4:(qq + 1) * 4, 0:HID_F],
                    in_=p4[:].rearrange("p (q c) -> p q c", c=66)[:, :, 0:HID_F],
                    func=AF.Copy)
                nc.vector.tensor_copy(
                    out=tbf.rearrange("p (q w) -> p q w", w=T1)[
                        :, qq * 4:(qq + 1) * 4, T1 - 2:T1],
                    in_=p4[:].rearrange("p (q c) -> p q c", c=66)[
                        :, :, HID_F:HID_F + 2])
            nc.sync.dma_start(
                out=H1[n0:n0 + 2048, :].rearrange("(q p) e -> p q e", p=P),
                in_=tb[:].rearrange("p (q e) -> p q e", e=E1))

        _agg_layer(nc, sb, psum, Ks, ixsb, H1[:], E1, HID_F, T1, b1rep,
                   w2sb, ident, h2lo)

    nc.compile()
    return nc


def _build_nc2(Ks):
    SUMK = sum(Ks)
    nc = bacc.Bacc("TRN2", target_bir_lowering=False, debug=False,
                   enable_asserts=False, num_devices=CORES,
                   num_swdge_queues=2)
    h2t = nc.dram_tensor("h2t", [GSLOTS + 1, E2], bf, kind="ExternalInput")
    b2d = nc.dram_tensor("b2d", [1, OUT_F], bf, kind="ExternalInput")
    ixd = nc.dram_tensor("ixd", [P, SUMK], i32, kind="ExternalInput")
    outp = nc.dram_tensor("outp", [P, NBLK * OUT_F], f32, kind="ExternalOutput")

    with ExitStack() as ctx:
        tc = ctx.enter_context(tile.TileContext(nc))
        consts, psum, ones1, ixsb = _mk_common(nc, tc, ctx, SUMK, ixd)
        b2rep = _rep_bias(nc, consts, psum, ones1, b2d, OUT_F, "pb2")
        sb = ctx.enter_context(tc.tile_pool(name="sb", bufs=3))
        _agg_layer(nc, sb, psum, Ks, ixsb, h2t.ap(), E2, OUT_F, T2, b2rep,
                   None, None, outp)

    nc.compile()
    return nc


# ------------------------------------------------------------------- kernel
def kernel(x, edge_index, W1, att_src1, att_dst1, b1, W2, att_src2, att_dst2,
           b2, _trace=False):
    global LAST_RESULT
    x = np.asarray(x, dtype=np.float32)
    W1 = np.asarray(W1, dtype=np.float32)
    W2 = np.asarray(W2, dtype=np.float32)

    Ks, order, idx = _host_prep(np.asarray(edge_index))

    key = tuple(Ks)
    if key not in _CACHE:
        _CACHE[key] = (_build_nc1(Ks), _build_nc2(Ks))
    nc1, nc2 = _CACHE[key]

    bfnp = ml_dtypes.bfloat16
    xp = np.zeros((GSLOTS, IN_F), dtype=np.float32)
    xp[:N] = x[order]
    xTp = np.ascontiguousarray(xp.T).astype(bfnp)
    w1ext = np.concatenate(
        [W1, (W1 @ np.asarray(att_src1, np.float32))[:, None],
         (W1 @ np.asarray(att_dst1, np.float32))[:, None]], axis=1).astype(bfnp)
    w2ext = np.concatenate(
        [W2, (W2 @ np.asarray(att_src2, np.float32))[:, None],
         (W2 @ np.asarray(att_dst2, np.float32))[:, None]], axis=1).astype(bfnp)
    b1a = np.asarray(b1, np.float32)[None, :].astype(bfnp)
    b2a = np.asarray(b2, np.float32)[None, :].astype(bfnp)

    in1 = [{"xt": xTp, "w1e": w1ext, "w2e": w2ext, "b1d": b1a, "ixd": idx[c]}
           for c in range(CORES)]
    r1 = run_bass_kernel_spmd(nc1, in1, core_ids=list(range(CORES)),
                              trace=_trace)

    h2full = np.empty((GSLOTS + 1, E2), dtype=bfnp)
    h2v = h2full[:GSLOTS].reshape(NBLK, CORES, P, E2)
    for c in range(CORES):
        h2v[:, c] = (np.asarray(r1.results[c]["h2lo"])
                     .reshape(P, NBLK, E2).transpose(1, 0, 2))
    sent = np.zeros(E2, dtype=bfnp)
    sent.view(np.float32)[T2 - 2:T2] = -1e30
    h2full[GSLOTS] = sent

    in2 = [{"h2t": h2full, "b2d": b2a, "ixd": idx[c]} for c in range(CORES)]
    r2 = run_bass_kernel_spmd(nc2, in2, core_ids=list(range(CORES)),
                              trace=_trace)
    LAST_RESULT = (r1, r2)

    big = np.empty((NBLK, CORES, P, OUT_F), dtype=np.float32)
    for c in range(CORES):
        big[:, c] = (np.asarray(r2.results[c]["outp"])
                     .reshape(P, NBLK, OUT_F).transpose(1, 0, 2))
    out = np.empty((N, OUT_F), dtype=np.float32)
    out[order] = big.reshape(GSLOTS, OUT_F)[:N]
    return out


# revision 9
# speedup vs baseline: 1.1691x; 1.1691x over previous
"""2-layer GAT on 8 trn2 NeuronCores — v3.

Same math as the staged baseline, restructured for fewer DMA instructions:
 - Host: degree-sort nodes; position space of 98*1024 slots; block j*8+c
   belongs to core c. One per-core index matrix [128, sum(K_j)] i32 (blocks
   side by side in columns, self loop at slot 0, pad -> sentinel row) serves
   BOTH layers (both tables in position space, sentinel appended at the end).
 - prog1: H1 table [GSLOTS+1, 68] bf16 built in 2048-node groups (1 load +
   1 interleaved write per group), index matrix preloaded to SBUF in ONE DMA,
   per-block gathers via per-slot [P,1] indirect DMA (the only ucode-correct
   form), softmax-weighted aggregation per partition, W2 projection, h2 shard
   written partition-major [128, 98*36].
 - Host reassembles the full layer-2 table (free), appends sentinel row.
 - prog2: same aggregation + final row softmax, output [128, 98*32] f32.
"""

import sys
from contextlib import ExitStack

import numpy as np

sys.path.insert(0, "/opt/trn_rl_repo")

import ml_dtypes  # noqa: E402

import concourse.bass as bass  # noqa: E402
import concourse.bacc as bacc  # noqa: E402
import concourse.tile as tile  # noqa: E402
from concourse import mybir  # noqa: E402
from concourse.bass_utils import run_bass_kernel_spmd  # noqa: E402
from concourse.masks import make_identity  # noqa: E402

N = 100000
E = 3200000
IN_F, HID_F, OUT_F = 128, 64, 32
NEG = 0.2
CORES = 8
P = 128
NBLK = 98
GSLOTS = NBLK * CORES * P   # 100352 global position slots
SENT = GSLOTS               # sentinel row in both tables
E1 = HID_F + 4              # 68 bf16: h(64) | gs f32 | gd f32
E2 = OUT_F + 4              # 36 bf16
T1 = E1 // 2
T2 = E2 // 2

bf = mybir.dt.bfloat16
f32 = mybir.dt.float32
i32 = mybir.dt.int32
AF = mybir.ActivationFunctionType
OP = mybir.AluOpType
AX = mybir.AxisListType

LAST_RESULT = None
_CACHE = {}


# ----------------------------------------------------------------- host prep
def _host_prep(edge_index):
    src = np.asarray(edge_index[0], dtype=np.int64)
    dst = np.asarray(edge_index[1], dtype=np.int64)
    deg = np.bincount(dst, minlength=N).astype(np.int64) + 1
    order = np.argsort(-deg, kind="stable")
    degs = deg[order]
    Ks = [int(degs[j * CORES * P]) for j in range(NBLK)]
    pos_of_node = np.empty(N, dtype=np.int64)
    pos_of_node[order] = np.arange(N)

    eorder = np.argsort(dst, kind="stable")
    ss = src[eorder]
    ds = dst[eorder]
    counts = np.bincount(dst, minlength=N)
    starts = np.zeros(N, dtype=np.int64)
    starts[1:] = np.cumsum(counts)[:-1]

    Kmax = Ks[0]
    Mpos = np.full((GSLOTS, Kmax), SENT, dtype=np.int32)
    Mpos[:N, 0] = np.arange(N, dtype=np.int32)
    slot_k = (np.arange(E) - starts[ds] + 1).astype(np.int64)
    Mpos[pos_of_node[ds], slot_k] = pos_of_node[ss].astype(np.int32)

    Mv = Mpos.reshape(NBLK, CORES, P, Kmax)
    idx = np.empty((CORES, P, sum(Ks)), dtype=np.int32)
    for c in range(CORES):
        idx[c] = np.concatenate(
            [Mv[j, c, :, :Ks[j]] for j in range(NBLK)], axis=1)
    return Ks, order, idx


# ------------------------------------------------------------- device pieces
def _mk_common(nc, tc, ctx, SUMK, ixd):
    consts = ctx.enter_context(tc.tile_pool(name="consts", bufs=1))
    psum = ctx.enter_context(tc.tile_pool(name="psum", bufs=2, space="PSUM"))
    ones1 = consts.tile([1, P], bf)
    nc.gpsimd.memset(ones1[:], 1.0)
    ixsb = consts.tile([P, SUMK], i32)
    nc.sync.dma_start(out=ixsb[:], in_=ixd.ap())
    return consts, psum, ones1, ixsb


def _rep_bias(nc, consts, psum, ones1, bd, fw, tag):
    br = consts.tile([1, fw], bf)
    nc.sync.dma_start(out=br[:], in_=bd.ap())
    brep = consts.tile([P, fw], f32)
    pb = psum.tile([P, fw], f32, tag=tag)
    nc.tensor.matmul(out=pb[:], lhsT=ones1[:], rhs=br[:], start=True, stop=True)
    nc.vector.tensor_copy(out=brep[:], in_=pb[:])
    return brep


def _agg_layer(nc, sb, psum, Ks, ixsb, table_ap, ew, fw, tw, brep,
               wnext, ident, outdram):
    cum = np.zeros(NBLK + 1, dtype=np.int64)
    cum[1:] = np.cumsum(Ks)
    Kmax = Ks[0]
    for j in range(NBLK):
        K = Ks[j]
        c0 = int(cum[j])
        hg = sb.tile([P, K * ew], bf, tag="hg", padded_shape=[P, Kmax * ew])
        for k in range(K):
            nc.gpsimd.indirect_dma_start(
                out=hg[:, k * ew:(k + 1) * ew], out_offset=None,
                in_=table_ap,
                in_offset=bass.IndirectOffsetOnAxis(
                    ap=ixsb[:, c0 + k:c0 + k + 1], axis=0))
        hgf = hg[:].bitcast(f32)
        z = sb.tile([P, K], f32, tag="z", padded_shape=[P, Kmax])
        nc.vector.tensor_scalar(
            out=z[:].rearrange("p (k o) -> p k o", o=1),
            in0=hgf.rearrange("p (k t) -> p k t", t=tw)[:, :, tw - 2:tw - 1],
            scalar1=hgf[:, tw - 1:tw], scalar2=None, op0=OP.add)
        zl = sb.tile([P, K], f32, tag="zl", padded_shape=[P, Kmax])
        nc.vector.scalar_tensor_tensor(
            out=zl[:], in0=z[:], scalar=NEG, in1=z[:],
            op0=OP.mult, op1=OP.max)
        ez = sb.tile([P, K], bf, tag="ez", padded_shape=[P, Kmax])
        den = sb.tile([P, 1], f32, tag="den")
        nc.scalar.activation(out=ez[:], in_=zl[:], func=AF.Exp,
                             accum_out=den[:])
        r = sb.tile([P, 1], f32, tag="r")
        nc.vector.reciprocal(out=r[:], in_=den[:])
        tmp = sb.tile([P, fw * K], bf, tag="tmp", padded_shape=[P, fw * Kmax])
        hg3 = hg[:].rearrange("p (k e) -> p e k", e=ew)[:, 0:fw, :]
        ez3 = ez[:].rearrange("p (k o) -> p o k", o=1).to_broadcast([P, fw, K])
        nc.vector.tensor_tensor(
            out=tmp[:].rearrange("p (f k) -> p f k", k=K),
            in0=hg3, in1=ez3, op=OP.mult)
        num = sb.tile([P, fw], f32, tag="num")
        nc.vector.tensor_reduce(
            out=num[:], in_=tmp[:].rearrange("p (f k) -> p f k", k=K),
            axis=AX.X, op=OP.add)
        o1 = sb.tile([P, fw], f32, tag="o1")
        nc.vector.scalar_tensor_tensor(
            out=o1[:], in0=num[:], scalar=r[:], in1=brep[:],
            op0=OP.mult, op1=OP.add)
        if wnext is not None:
            o1b = sb.tile([P, fw], bf, tag="o1b")
            nc.scalar.activation(out=o1b[:], in_=o1[:], func=AF.Relu)
            pt = psum.tile([fw, P], bf, tag="pt")
            nc.tensor.transpose(out=pt[:], in_=o1b[:], identity=ident[:])
            o1T = sb.tile([fw, P], bf, tag="o1T")
            nc.scalar.activation(out=o1T[:], in_=pt[:], func=AF.Copy)
            p34 = psum.tile([P, OUT_F + 2], f32, tag="p34")
            nc.tensor.matmul(out=p34[:], lhsT=o1T[:], rhs=wnext[:],
                             start=True, stop=True)
            th2 = sb.tile([P, E2], bf, tag="th2")
            nc.scalar.activation(out=th2[:, 0:OUT_F], in_=p34[:, 0:OUT_F],
                                 func=AF.Copy)
            nc.vector.tensor_copy(
                out=th2[:, OUT_F:OUT_F + 4].bitcast(f32),
                in_=p34[:, OUT_F:OUT_F + 2])
            nc.sync.dma_start(out=outdram.ap()[:, j * E2:(j + 1) * E2],
                              in_=th2[:])
        else:
            negm = sb.tile([P, 1], f32, tag="negm")
            nc.vector.tensor_reduce(out=negm[:], in_=o1[:], axis=AX.X,
                                    op=OP.max, negate=True)
            e2 = sb.tile([P, fw], f32, tag="e2")
            nc.scalar.activation(out=e2[:], in_=o1[:], func=AF.Exp,
                                 bias=negm[:])
            ssum = sb.tile([P, 1], f32, tag="ssum")
            nc.vector.tensor_reduce(out=ssum[:], in_=e2[:], axis=AX.X,
                                    op=OP.add)
            rs = sb.tile([P, 1], f32, tag="rs")
            nc.vector.reciprocal(out=rs[:], in_=ssum[:])
            of = sb.tile([P, fw], f32, tag="of")
            nc.vector.tensor_scalar(out=of[:], in0=e2[:], scalar1=rs[:],
                                    scalar2=None, op0=OP.mult)
            nc.sync.dma_start(out=outdram.ap()[:, j * fw:(j + 1) * fw],
                              in_=of[:])


def _build_nc1(Ks):
    SUMK = sum(Ks)
    nc = bacc.Bacc("TRN2", target_bir_lowering=False, debug=False,
                   enable_asserts=False, num_devices=CORES)
    xT = nc.dram_tensor("xt", [IN_F, GSLOTS], bf, kind="ExternalInput")
    w1e = nc.dram_tensor("w1e", [IN_F, HID_F + 2], bf, kind="ExternalInput")
    w2e = nc.dram_tensor("w2e", [HID_F, OUT_F + 2], bf, kind="ExternalInput")
    b1d = nc.dram_tensor("b1d", [1, HID_F], bf, kind="ExternalInput")
    ixd = nc.dram_tensor("ixd", [P, SUMK], i32, kind="ExternalInput")
    h2lo = nc.dram_tensor("h2lo", [P, NBLK * E2], bf, kind="ExternalOutput")

    with ExitStack() as ctx:
        tc = ctx.enter_context(tile.TileContext(nc))
        dram = ctx.enter_context(tc.tile_pool(name="dram", bufs=1, space="DRAM"))
        H1 = dram.tile([GSLOTS + 1, E1], bf)
        consts, psum, ones1, ixsb = _mk_common(nc, tc, ctx, SUMK, ixd)
        ident = consts.tile([P, P], bf)
        make_identity(nc, ident[:])
        w1sb = consts.tile([IN_F, HID_F + 2], bf)
        nc.sync.dma_start(out=w1sb[:], in_=w1e.ap())
        w2sb = consts.tile([HID_F, OUT_F + 2], bf)
        nc.sync.dma_start(out=w2sb[:], in_=w2e.ap())
        b1rep = _rep_bias(nc, consts, psum, ones1, b1d, HID_F, "pb1")

        s1 = consts.tile([1, E1], bf)
        nc.gpsimd.memset(s1[:], 0.0)
        nc.gpsimd.memset(s1[:, HID_F:HID_F + 4].bitcast(f32), -1e30)
        nc.sync.dma_start(out=H1[GSLOTS:GSLOTS + 1, :], in_=s1[:])

        sb = ctx.enter_context(tc.tile_pool(name="sb", bufs=3))

        NGRP = GSLOTS // 2048  # 49
        for g in range(NGRP):
            n0 = g * 2048
            xt_t = sb.tile([IN_F, 2048], bf, tag="xt")
            nc.sync.dma_start(out=xt_t[:], in_=xT.ap()[:, n0:n0 + 2048])
            tb = sb.tile([P, 16 * E1], bf, tag="tb")
            tbf = tb[:].bitcast(f32)
            for qq in range(4):
                p4 = psum.tile([P, 4 * (HID_F + 2)], f32, tag="p4")
                for qi in range(4):
                    q = qq * 4 + qi
                    nc.tensor.matmul(
                        out=p4[:, qi * 66:(qi + 1) * 66],
                        lhsT=xt_t[:, q * P:(q + 1) * P],
                        rhs=w1sb[:], start=True, stop=True)
                nc.scalar.activation(
                    out=tb[:].rearrange("p (q e) -> p q e", e=E1)[
                        :, qq * 4:(qq + 1) * 4, 0:HID_F],
                    in_=p4[:].rearrange("p (q c) -> p q c", c=66)[:, :, 0:HID_F],
                    func=AF.Copy)
                nc.vector.tensor_copy(
                    out=tbf.rearrange("p (q w) -> p q w", w=T1)[
                        :, qq * 4:(qq + 1) * 4, T1 - 2:T1],
                    in_=p4[:].rearrange("p (q c) -> p q c", c=66)[
                        :, :, HID_F:HID_F + 2])
            nc.sync.dma_start(
                out=H1[n0:n0 + 2048, :].rearrange("(q p) e -> p q e", p=P),
                in_=tb[:].rearrange("p (q e) -> p q e", e=E1))

        _agg_layer(nc, sb, psum, Ks, ixsb, H1[:], E1, HID_F, T1, b1rep,
                   w2sb, ident, h2lo)

    nc.compile()
    return nc


def _build_nc2(Ks):
    SUMK = sum(Ks)
    nc = bacc.Bacc("TRN2", target_bir_lowering=False, debug=False,
                   enable_asserts=False, num_devices=CORES)
    h2t = nc.dram_tensor("h2t", [GSLOTS + 1, E2], bf, kind="ExternalInput")
    b2d = nc.dram_tensor("b2d", [1, OUT_F], bf, kind="ExternalInput")
    ixd = nc.dram_tensor("ixd", [P, SUMK], i32, kind="ExternalInput")
    outp = nc.dram_tensor("outp", [P, NBLK * OUT_F], f32, kind="ExternalOutput")

    with ExitStack() as ctx:
        tc = ctx.enter_context(tile.TileContext(nc))
        consts, psum, ones1, ixsb = _mk_common(nc, tc, ctx, SUMK, ixd)
        b2rep = _rep_bias(nc, consts, psum, ones1, b2d, OUT_F, "pb2")
        sb = ctx.enter_context(tc.tile_pool(name="sb", bufs=3))
        _agg_layer(nc, sb, psum, Ks, ixsb, h2t.ap(), E2, OUT_F, T2, b2rep,
                   None, None, outp)

    nc.compile()
    return nc


# ------------------------------------------------------------------- kernel
def kernel(x, edge_index, W1, att_src1, att_dst1, b1, W2, att_src2, att_dst2,
           b2, _trace=False):
    global LAST_RESULT
    x = np.asarray(x, dtype=np.float32)
    W1 = np.asarray(W1, dtype=np.float32)
    W2 = np.asarray(W2, dtype=np.float32)

    Ks, order, idx = _host_prep(np.asarray(edge_index))

    key = tuple(Ks)
    if key not in _CACHE:
        _CACHE[key] = (_build_nc1(Ks), _build_nc2(Ks))
    nc1, nc2 = _CACHE[key]

    bfnp = ml_dtypes.bfloat16
    xp = np.zeros((GSLOTS, IN_F), dtype=np.float32)
    xp[:N] = x[order]
    xTp = np.ascontiguousarray(xp.T).astype(bfnp)
    w1ext = np.concatenate(
        [W1, (W1 @ np.asarray(att_src1, np.float32))[:, None],
         (W1 @ np.asarray(att_dst1, np.float32))[:, None]], axis=1).astype(bfnp)
    w2ext = np.concatenate(
        [W2, (W2 @ np.asarray(att_src2, np.float32))[:, None],
         (W2 @ np.asarray(att_dst2, np.float32))[:, None]], axis=1).astype(bfnp)
    b1a = np.asarray(b1, np.float32)[None, :].astype(bfnp)
    b2a = np.asarray(b2, np.float32)[None, :].astype(bfnp)

    in1 = [{"xt": xTp, "w1e": w1ext, "w2e": w2ext, "b1d": b1a, "ixd": idx[c]}
           for c in range(CORES)]
    r1 = run_bass_kernel_spmd(nc1, in1, core_ids=list(range(CORES)),
                              trace=_trace)

    h2full = np.empty((GSLOTS + 1, E2), dtype=bfnp)
    h2v = h2full[:GSLOTS].reshape(NBLK, CORES, P, E2)
    for c in range(CORES):
        h2v[:, c] = (np.asarray(r1.results[c]["h2lo"])
                     .reshape(P, NBLK, E2).transpose(1, 0, 2))
    sent = np.zeros(E2, dtype=bfnp)
    sent.view(np.float32)[T2 - 2:T2] = -1e30
    h2full[GSLOTS] = sent

    in2 = [{"h2t": h2full, "b2d": b2a, "ixd": idx[c]} for c in range(CORES)]
    r2 = run_bass_kernel_spmd(nc2, in2, core_ids=list(range(CORES)),
                              trace=_trace)
    LAST_RESULT = (r1, r2)

    big = np.empty((NBLK, CORES, P, OUT_F), dtype=np.float32)
    for c in range(CORES):
        big[:, c] = (np.asarray(r2.results[c]["outp"])
                     .reshape(P, NBLK, OUT_F).transpose(1, 0, 2))
    out = np.empty((N, OUT_F), dtype=np.float32)
    out[order] = big.reshape(GSLOTS, OUT_F)[:N]
    return out


# revision 10
# speedup vs baseline: 1.1954x; 1.0225x over previous
"""2-layer GAT on 8 trn2 NeuronCores — v3.

Same math as the staged baseline, restructured for fewer DMA instructions:
 - Host: degree-sort nodes; position space of 98*1024 slots; block j*8+c
   belongs to core c. One per-core index matrix [128, sum(K_j)] i32 (blocks
   side by side in columns, self loop at slot 0, pad -> sentinel row) serves
   BOTH layers (both tables in position space, sentinel appended at the end).
 - prog1: H1 table [GSLOTS+1, 68] bf16 built in 2048-node groups (1 load +
   1 interleaved write per group), index matrix preloaded to SBUF in ONE DMA,
   per-block gathers via per-slot [P,1] indirect DMA (the only ucode-correct
   form), softmax-weighted aggregation per partition, W2 projection, h2 shard
   written partition-major [128, 98*36].
 - Host reassembles the full layer-2 table (free), appends sentinel row.
 - prog2: same aggregation + final row softmax, output [128, 98*32] f32.
"""

import sys
from contextlib import ExitStack

import numpy as np

sys.path.insert(0, "/opt/trn_rl_repo")

import ml_dtypes  # noqa: E402

import concourse.bass as bass  # noqa: E402
import concourse.bacc as bacc  # noqa: E402
import concourse.tile as tile  # noqa: E402
from concourse import mybir  # noqa: E402
from concourse.bass_utils import run_bass_kernel_spmd  # noqa: E402
from concourse.masks import make_identity  # noqa: E402

N = 100000
E = 3200000
IN_F, HID_F, OUT_F = 128, 64, 32
NEG = 0.2
CORES = 8
P = 128
NBLK = 98
GSLOTS = NBLK * CORES * P   # 100352 global position slots
SENT = GSLOTS               # sentinel row in both tables
E1 = HID_F + 4              # 68 bf16: h(64) | gs f32 | gd f32
E2 = OUT_F + 4              # 36 bf16
T1 = E1 // 2
T2 = E2 // 2

bf = mybir.dt.bfloat16
f32 = mybir.dt.float32
i32 = mybir.dt.int32
AF = mybir.ActivationFunctionType
OP = mybir.AluOpType
AX = mybir.AxisListType

LAST_RESULT = None
_CACHE = {}


# ----------------------------------------------------------------- host prep
def _host_prep(edge_index):
    src = np.asarray(edge_index[0], dtype=np.int64)
    dst = np.asarray(edge_index[1], dtype=np.int64)
    deg = np.bincount(dst, minlength=N).astype(np.int64) + 1
    order = np.argsort(-deg, kind="stable")
    degs = deg[order]
    Ks = [int(degs[j * CORES * P]) for j in range(NBLK)]
    pos_of_node = np.empty(N, dtype=np.int64)
    pos_of_node[order] = np.arange(N)

    eorder = np.argsort(dst, kind="stable")
    ss = src[eorder]
    ds = dst[eorder]
    counts = np.bincount(dst, minlength=N)
    starts = np.zeros(N, dtype=np.int64)
    starts[1:] = np.cumsum(counts)[:-1]

    Kmax = Ks[0]
    Mpos = np.full((GSLOTS, Kmax), SENT, dtype=np.int32)
    Mpos[:N, 0] = np.arange(N, dtype=np.int32)
    slot_k = (np.arange(E) - starts[ds] + 1).astype(np.int64)
    Mpos[pos_of_node[ds], slot_k] = pos_of_node[ss].astype(np.int32)

    Mv = Mpos.reshape(NBLK, CORES, P, Kmax)
    idx = np.empty((CORES, P, sum(Ks)), dtype=np.int32)
    for c in range(CORES):
        idx[c] = np.concatenate(
            [Mv[j, c, :, :Ks[j]] for j in range(NBLK)], axis=1)
    return Ks, order, idx


# ------------------------------------------------------------- device pieces
def _mk_common(nc, tc, ctx, SUMK, ixd):
    consts = ctx.enter_context(tc.tile_pool(name="consts", bufs=1))
    psum = ctx.enter_context(tc.tile_pool(name="psum", bufs=2, space="PSUM"))
    ones1 = consts.tile([1, P], bf)
    nc.gpsimd.memset(ones1[:], 1.0)
    ixsb = consts.tile([P, SUMK], i32)
    nc.sync.dma_start(out=ixsb[:], in_=ixd.ap())
    return consts, psum, ones1, ixsb


def _rep_bias(nc, consts, psum, ones1, bd, fw, tag):
    br = consts.tile([1, fw], bf)
    nc.sync.dma_start(out=br[:], in_=bd.ap())
    brep = consts.tile([P, fw], f32)
    pb = psum.tile([P, fw], f32, tag=tag)
    nc.tensor.matmul(out=pb[:], lhsT=ones1[:], rhs=br[:], start=True, stop=True)
    nc.vector.tensor_copy(out=brep[:], in_=pb[:])
    return brep


def _agg_layer(nc, sb, psum, Ks, ixsb, table_ap, ew, fw, tw, brep,
               wnext, ident, outdram, selfsb=None):
    cum = np.zeros(NBLK + 1, dtype=np.int64)
    cum[1:] = np.cumsum(Ks)
    Kmax = Ks[0]
    for j in range(NBLK):
        K = Ks[j]
        c0 = int(cum[j])
        hg = sb.tile([P, K * ew], bf, tag="hg", padded_shape=[P, Kmax * ew])
        kstart = 0
        if selfsb is not None:
            nc.scalar.activation(out=hg[:, 0:ew],
                                 in_=selfsb[:, j * ew:(j + 1) * ew],
                                 func=AF.Copy)
            kstart = 1
        for k in range(kstart, K):
            nc.gpsimd.indirect_dma_start(
                out=hg[:, k * ew:(k + 1) * ew], out_offset=None,
                in_=table_ap,
                in_offset=bass.IndirectOffsetOnAxis(
                    ap=ixsb[:, c0 + k:c0 + k + 1], axis=0))
        hgf = hg[:].bitcast(f32)
        z = sb.tile([P, K], f32, tag="z", padded_shape=[P, Kmax])
        nc.vector.tensor_scalar(
            out=z[:].rearrange("p (k o) -> p k o", o=1),
            in0=hgf.rearrange("p (k t) -> p k t", t=tw)[:, :, tw - 2:tw - 1],
            scalar1=hgf[:, tw - 1:tw], scalar2=None, op0=OP.add)
        zl = sb.tile([P, K], f32, tag="zl", padded_shape=[P, Kmax])
        nc.vector.scalar_tensor_tensor(
            out=zl[:], in0=z[:], scalar=NEG, in1=z[:],
            op0=OP.mult, op1=OP.max)
        ez = sb.tile([P, K], bf, tag="ez", padded_shape=[P, Kmax])
        den = sb.tile([P, 1], f32, tag="den")
        nc.scalar.activation(out=ez[:], in_=zl[:], func=AF.Exp,
                             accum_out=den[:])
        r = sb.tile([P, 1], f32, tag="r")
        nc.vector.reciprocal(out=r[:], in_=den[:])
        tmp = sb.tile([P, fw * K], bf, tag="tmp", padded_shape=[P, fw * Kmax])
        hg3 = hg[:].rearrange("p (k e) -> p e k", e=ew)[:, 0:fw, :]
        ez3 = ez[:].rearrange("p (k o) -> p o k", o=1).to_broadcast([P, fw, K])
        nc.vector.tensor_tensor(
            out=tmp[:].rearrange("p (f k) -> p f k", k=K),
            in0=hg3, in1=ez3, op=OP.mult)
        num = sb.tile([P, fw], f32, tag="num")
        nc.vector.tensor_reduce(
            out=num[:], in_=tmp[:].rearrange("p (f k) -> p f k", k=K),
            axis=AX.X, op=OP.add)
        o1 = sb.tile([P, fw], f32, tag="o1")
        nc.vector.scalar_tensor_tensor(
            out=o1[:], in0=num[:], scalar=r[:], in1=brep[:],
            op0=OP.mult, op1=OP.add)
        if wnext is not None:
            o1b = sb.tile([P, fw], bf, tag="o1b")
            nc.scalar.activation(out=o1b[:], in_=o1[:], func=AF.Relu)
            pt = psum.tile([fw, P], bf, tag="pt")
            nc.tensor.transpose(out=pt[:], in_=o1b[:], identity=ident[:])
            o1T = sb.tile([fw, P], bf, tag="o1T")
            nc.scalar.activation(out=o1T[:], in_=pt[:], func=AF.Copy)
            p34 = psum.tile([P, OUT_F + 2], f32, tag="p34")
            nc.tensor.matmul(out=p34[:], lhsT=o1T[:], rhs=wnext[:],
                             start=True, stop=True)
            th2 = sb.tile([P, E2], bf, tag="th2")
            nc.scalar.activation(out=th2[:, 0:OUT_F], in_=p34[:, 0:OUT_F],
                                 func=AF.Copy)
            nc.vector.tensor_copy(
                out=th2[:, OUT_F:OUT_F + 4].bitcast(f32),
                in_=p34[:, OUT_F:OUT_F + 2])
            nc.sync.dma_start(out=outdram.ap()[:, j * E2:(j + 1) * E2],
                              in_=th2[:])
        else:
            negm = sb.tile([P, 1], f32, tag="negm")
            nc.vector.tensor_reduce(out=negm[:], in_=o1[:], axis=AX.X,
                                    op=OP.max, negate=True)
            e2 = sb.tile([P, fw], f32, tag="e2")
            nc.scalar.activation(out=e2[:], in_=o1[:], func=AF.Exp,
                                 bias=negm[:])
            ssum = sb.tile([P, 1], f32, tag="ssum")
            nc.vector.tensor_reduce(out=ssum[:], in_=e2[:], axis=AX.X,
                                    op=OP.add)
            rs = sb.tile([P, 1], f32, tag="rs")
            nc.vector.reciprocal(out=rs[:], in_=ssum[:])
            of = sb.tile([P, fw], f32, tag="of")
            nc.vector.tensor_scalar(out=of[:], in0=e2[:], scalar1=rs[:],
                                    scalar2=None, op0=OP.mult)
            nc.sync.dma_start(out=outdram.ap()[:, j * fw:(j + 1) * fw],
                              in_=of[:])


def _build_nc1(Ks):
    SUMK = sum(Ks)
    nc = bacc.Bacc("TRN2", target_bir_lowering=False, debug=False,
                   enable_asserts=False, num_devices=CORES)
    xT = nc.dram_tensor("xt", [IN_F, GSLOTS], bf, kind="ExternalInput")
    w1e = nc.dram_tensor("w1e", [IN_F, HID_F + 2], bf, kind="ExternalInput")
    w2e = nc.dram_tensor("w2e", [HID_F, OUT_F + 2], bf, kind="ExternalInput")
    b1d = nc.dram_tensor("b1d", [1, HID_F], bf, kind="ExternalInput")
    ixd = nc.dram_tensor("ixd", [P, SUMK], i32, kind="ExternalInput")
    h2lo = nc.dram_tensor("h2lo", [P, NBLK * E2], bf, kind="ExternalOutput")

    with ExitStack() as ctx:
        tc = ctx.enter_context(tile.TileContext(nc))
        dram = ctx.enter_context(tc.tile_pool(name="dram", bufs=1, space="DRAM"))
        H1 = dram.tile([GSLOTS + 1, E1], bf)
        consts, psum, ones1, ixsb = _mk_common(nc, tc, ctx, SUMK, ixd)
        ident = consts.tile([P, P], bf)
        make_identity(nc, ident[:])
        w1sb = consts.tile([IN_F, HID_F + 2], bf)
        nc.sync.dma_start(out=w1sb[:], in_=w1e.ap())
        w2sb = consts.tile([HID_F, OUT_F + 2], bf)
        nc.sync.dma_start(out=w2sb[:], in_=w2e.ap())
        b1rep = _rep_bias(nc, consts, psum, ones1, b1d, HID_F, "pb1")

        s1 = consts.tile([1, E1], bf)
        nc.gpsimd.memset(s1[:], 0.0)
        nc.gpsimd.memset(s1[:, HID_F:HID_F + 4].bitcast(f32), -1e30)
        nc.sync.dma_start(out=H1[GSLOTS:GSLOTS + 1, :], in_=s1[:])

        sb = ctx.enter_context(tc.tile_pool(name="sb", bufs=3))

        NGRP = GSLOTS // 2048  # 49
        for g in range(NGRP):
            n0 = g * 2048
            xt_t = sb.tile([IN_F, 2048], bf, tag="xt")
            nc.sync.dma_start(out=xt_t[:], in_=xT.ap()[:, n0:n0 + 2048])
            tb = sb.tile([P, 16 * E1], bf, tag="tb")
            tbf = tb[:].bitcast(f32)
            for qq in range(4):
                p4 = psum.tile([P, 4 * (HID_F + 2)], f32, tag="p4")
                for qi in range(4):
                    q = qq * 4 + qi
                    nc.tensor.matmul(
                        out=p4[:, qi * 66:(qi + 1) * 66],
                        lhsT=xt_t[:, q * P:(q + 1) * P],
                        rhs=w1sb[:], start=True, stop=True)
                nc.scalar.activation(
                    out=tb[:].rearrange("p (q e) -> p q e", e=E1)[
                        :, qq * 4:(qq + 1) * 4, 0:HID_F],
                    in_=p4[:].rearrange("p (q c) -> p q c", c=66)[:, :, 0:HID_F],
                    func=AF.Copy)
                nc.vector.tensor_copy(
                    out=tbf.rearrange("p (q w) -> p q w", w=T1)[
                        :, qq * 4:(qq + 1) * 4, T1 - 2:T1],
                    in_=p4[:].rearrange("p (q c) -> p q c", c=66)[
                        :, :, HID_F:HID_F + 2])
            nc.sync.dma_start(
                out=H1[n0:n0 + 2048, :].rearrange("(q p) e -> p q e", p=P),
                in_=tb[:].rearrange("p (q e) -> p q e", e=E1))

        _agg_layer(nc, sb, psum, Ks, ixsb, H1[:], E1, HID_F, T1, b1rep,
                   w2sb, ident, h2lo)

    nc.compile()
    return nc


def _build_nc2(Ks):
    SUMK = sum(Ks)
    nc = bacc.Bacc("TRN2", target_bir_lowering=False, debug=False,
                   enable_asserts=False, num_devices=CORES)
    h2t = nc.dram_tensor("h2t", [GSLOTS + 1, E2], bf, kind="ExternalInput")
    selfd = nc.dram_tensor("selfd", [P, NBLK * E2], bf, kind="ExternalInput")
    b2d = nc.dram_tensor("b2d", [1, OUT_F], bf, kind="ExternalInput")
    ixd = nc.dram_tensor("ixd", [P, SUMK], i32, kind="ExternalInput")
    outp = nc.dram_tensor("outp", [P, NBLK * OUT_F], f32, kind="ExternalOutput")

    with ExitStack() as ctx:
        tc = ctx.enter_context(tile.TileContext(nc))
        consts, psum, ones1, ixsb = _mk_common(nc, tc, ctx, SUMK, ixd)
        selfsb = consts.tile([P, NBLK * E2], bf)
        nc.sync.dma_start(out=selfsb[:], in_=selfd.ap())
        b2rep = _rep_bias(nc, consts, psum, ones1, b2d, OUT_F, "pb2")
        sb = ctx.enter_context(tc.tile_pool(name="sb", bufs=3))
        _agg_layer(nc, sb, psum, Ks, ixsb, h2t.ap(), E2, OUT_F, T2, b2rep,
                   None, None, outp, selfsb=selfsb)

    nc.compile()
    return nc


# ------------------------------------------------------------------- kernel
def kernel(x, edge_index, W1, att_src1, att_dst1, b1, W2, att_src2, att_dst2,
           b2, _trace=False):
    global LAST_RESULT
    x = np.asarray(x, dtype=np.float32)
    W1 = np.asarray(W1, dtype=np.float32)
    W2 = np.asarray(W2, dtype=np.float32)

    Ks, order, idx = _host_prep(np.asarray(edge_index))

    key = tuple(Ks)
    if key not in _CACHE:
        _CACHE[key] = (_build_nc1(Ks), _build_nc2(Ks))
    nc1, nc2 = _CACHE[key]

    bfnp = ml_dtypes.bfloat16
    xp = np.zeros((GSLOTS, IN_F), dtype=np.float32)
    xp[:N] = x[order]
    xTp = np.ascontiguousarray(xp.T).astype(bfnp)
    w1ext = np.concatenate(
        [W1, (W1 @ np.asarray(att_src1, np.float32))[:, None],
         (W1 @ np.asarray(att_dst1, np.float32))[:, None]], axis=1).astype(bfnp)
    w2ext = np.concatenate(
        [W2, (W2 @ np.asarray(att_src2, np.float32))[:, None],
         (W2 @ np.asarray(att_dst2, np.float32))[:, None]], axis=1).astype(bfnp)
    b1a = np.asarray(b1, np.float32)[None, :].astype(bfnp)
    b2a = np.asarray(b2, np.float32)[None, :].astype(bfnp)

    in1 = [{"xt": xTp, "w1e": w1ext, "w2e": w2ext, "b1d": b1a, "ixd": idx[c]}
           for c in range(CORES)]
    r1 = run_bass_kernel_spmd(nc1, in1, core_ids=list(range(CORES)),
                              trace=_trace)

    h2full = np.empty((GSLOTS + 1, E2), dtype=bfnp)
    h2v = h2full[:GSLOTS].reshape(NBLK, CORES, P, E2)
    for c in range(CORES):
        h2v[:, c] = (np.asarray(r1.results[c]["h2lo"])
                     .reshape(P, NBLK, E2).transpose(1, 0, 2))
    sent = np.zeros(E2, dtype=bfnp)
    sent.view(np.float32)[T2 - 2:T2] = -1e30
    h2full[GSLOTS] = sent

    h2v2 = h2full[:GSLOTS].reshape(NBLK, CORES, P, E2)
    in2 = [{"h2t": h2full, "b2d": b2a, "ixd": idx[c],
            "selfd": np.ascontiguousarray(
                h2v2[:, c].transpose(1, 0, 2)).reshape(P, NBLK * E2)}
           for c in range(CORES)]
    r2 = run_bass_kernel_spmd(nc2, in2, core_ids=list(range(CORES)),
                              trace=_trace)
    LAST_RESULT = (r1, r2)

    big = np.empty((NBLK, CORES, P, OUT_F), dtype=np.float32)
    for c in range(CORES):
        big[:, c] = (np.asarray(r2.results[c]["outp"])
                     .reshape(P, NBLK, OUT_F).transpose(1, 0, 2))
    out = np.empty((N, OUT_F), dtype=np.float32)
    out[order] = big.reshape(GSLOTS, OUT_F)[:N]
    return out
